# revision 2
# baseline (speedup 1.0000x reference)
"""ChebConv (K=5) x2 GNN decoder on 8 TRN2 NeuronCores — v2.

Key design points vs v1:
- fp16 node tables (rows padded to 128 halfs = 256B for the gather-elem
  constraint), fp16 weights/matmuls (FWL + 2x PE rate), f32 PSUM.
- dma_gather calls round-robin over all 4 SWDGE queues (4 Q7 core pairs +
  4 descriptor rings): measured 2.3us per 1024-idx call vs 8.6us on one
  queue. This was the v1 bottleneck (86% GpSimd occupancy).
- One-hot (edge->target, scaled by norm) matrices precomputed on host in
  fp16 and streamed from DRAM, removing all per-tile DVE work.
- Chebyshev / Clenshaw recurrence terms folded into the PE accumulation
  group via +-identity matmuls; psum evacuation on the Scalar engine.
- AllGather split into two chunks (each core's first 25 / last 24 blocks)
  so the first chunk ships while the second half of the hop computes, and
  the next hop's gathers start after only the matching chunk lands.
"""
import os
import sys

sys.path.insert(0, "/opt/trn_rl_repo")

import numpy as np


def _install_ntff_hook():
    import types

    if "antenv.axon_hooks" in sys.modules:
        return
    try:
        import antenv
        from trn_agent_boot.trn_boot import _ntff_profile_via_ctypes
    except Exception:
        return
    mod = types.ModuleType("antenv.axon_hooks")
    state = {"hook": None}
    mod.set_axon_ntff_profile_hook = lambda h: state.__setitem__("hook", h)
    mod.get_axon_ntff_profile_hook = lambda: state["hook"]
    sys.modules["antenv.axon_hooks"] = mod
    antenv.axon_hooks = mod
    try:
        hook = _ntff_profile_via_ctypes("/opt/axon/libaxon_pjrt.so")
        if hook is not None:
            mod.set_axon_ntff_profile_hook(hook)
    except Exception:
        pass


_install_ntff_hook()

import concourse.bass as bass
import concourse.bacc as bacc
import concourse.mybir as mybir
import concourse.tile as tile
from concourse.bass_utils import run_bass_kernel_spmd

F32 = mybir.dt.float32
F16 = mybir.dt.float16
I16 = mybir.dt.int16
ALU = mybir.AluOpType
ACTF = mybir.ActivationFunctionType

NCORES = 8
F = 64
FH = 256
K = 5
P = 128
NT = 6250            # owned real targets per core
NB = 49              # 128-blocks per core
NTP = NB * P         # 6272 padded rows per core
CA = 25              # chunk-A blocks
RA = CA * P          # 3200 chunk-A rows per core
RB = NTP - RA        # 3072 chunk-B rows per core
GA = NCORES * RA     # 25600 global chunk-A table rows
GB = NCORES * RB     # 24576
EF = 128             # padded fp16 row elements (256B)
BG = 8               # tiles per gather call (1024 idx HW cap)
NQ = 4               # SWDGE queues


# ----------------------------------------------------------------------------
# host-side preprocessing
# ----------------------------------------------------------------------------

def _preprocess(x, edge_index, w):
    n = x.shape[0]
    assert n == NCORES * NT

    row = np.asarray(edge_index[0], dtype=np.int64)
    col = np.asarray(edge_index[1], dtype=np.int64)
    w = np.asarray(w, dtype=np.float32)

    deg = np.zeros(n, np.float64)
    np.add.at(deg, row, w.astype(np.float64))
    dis = np.where(deg > 0, 1.0 / np.sqrt(np.maximum(deg, 1e-30)), 0.0)
    norm = (-dis[row] * w.astype(np.float64) * dis[col]).astype(np.float32)

    c_src = row // NT
    j_src = row % NT
    in_b = j_src >= RA

    core_t = col // NT
    loc = col % NT
    blk = loc // P
    cloc = (loc % P).astype(np.int64)

    # per-core 3-way source chunk: 0 = own core (local table, ready before
    # the AllGather lands -> bridges AG latency), 1 = global chunk A,
    # 2 = global chunk B
    edir = {}
    cnt = np.zeros((NCORES, NB, 3), np.int64)
    for c in range(NCORES):
        ei = np.nonzero(core_t == c)[0]
        own = c_src[ei] == c
        ch = np.where(own, 0, np.where(in_b[ei], 2, 1))
        key = blk[ei] * 3 + ch
        order = np.argsort(key, kind="stable")
        ei = ei[order]
        bounds = np.searchsorted(key[order], np.arange(3 * NB + 1))
        edir[c] = (ei, bounds)
        cnt[c] = (bounds[1:] - bounds[:-1]).reshape(NB, 3)

    ntile = np.maximum((cnt.max(axis=0) + P - 1) // P, 1)  # [NB, 3]
    nO, nA, nB_ = ntile[:, 0], ntile[:, 1], ntile[:, 2]
    tO_all = int(nO.sum())
    tA_all = int(nA.sum())
    tB_all = int(nB_.sum())
    t_all = tO_all + tA_all + tB_all
    o_start = np.concatenate([[0], np.cumsum(nO)])[:-1]
    a_start = tO_all + np.concatenate([[0], np.cumsum(nA)])[:-1]
    b_start = tO_all + tA_all + np.concatenate([[0], np.cumsum(nB_)])[:-1]
    starts3 = (o_start, a_start, b_start)

    per_core = []
    x16 = np.asarray(x, np.float32).astype(np.float16)
    for c in range(NCORES):
        ei, bounds = edir[c]
        idx_flat = np.zeros(t_all * P, np.int16)
        cloc_flat = np.zeros(t_all * P, np.int64)
        nrm_flat = np.zeros(t_all * P, np.float32)
        src_flat = np.zeros(t_all * P, np.int64)
        for tb in range(NB):
            for h in range(3):
                lo, hi = bounds[tb * 3 + h], bounds[tb * 3 + h + 1]
                sel = ei[lo:hi]
                o = starts3[h][tb] * P
                s = row[sel]
                if h == 0:
                    idx_flat[o : o + len(sel)] = (s % NT).astype(np.int16)
                elif h == 1:
                    idx_flat[o : o + len(sel)] = (
                        (s // NT) * RA + (s % NT)
                    ).astype(np.int16)
                else:
                    idx_flat[o : o + len(sel)] = (
                        (s // NT) * RB + (s % NT - RA)
                    ).astype(np.int16)
                cloc_flat[o : o + len(sel)] = cloc[sel]
                nrm_flat[o : o + len(sel)] = norm[sel]
                src_flat[o : o + len(sel)] = s
        iw = idx_flat.reshape(t_all, 8, 16).transpose(2, 0, 1).reshape(16, t_all * 8)
        idx_w = np.tile(iw, (8, 1))
        # one-hot masks [128 edge-slot partitions, t_all*128 target cols],
        # tile-columns permuted into per-block consumption order so mask
        # chunks stream as single contiguous DMAs.
        cons = []
        for tb in range(NB):
            for h in range(3):
                cons += [starts3[h][tb] + j for j in range(ntile[tb, h])]
        m = np.zeros((t_all * P, P), np.float16)
        m[np.arange(t_all * P), cloc_flat] = nrm_flat
        m = m.reshape(t_all, P, P).transpose(1, 0, 2)  # [P, t_all, P]
        mask1 = np.ascontiguousarray(m[:, cons, :]).reshape(P, t_all * P)
        mask2 = (mask1.astype(np.float32) * 2.0).astype(np.float16)
        xg = x16[src_flat]  # [t_all*P, F] fp16, junk on pad slots (mask=0)
        xg = xg.reshape(t_all, P, F).transpose(1, 0, 2)  # [P, t_all, F]
        xg = np.ascontiguousarray(xg[:, cons, :]).reshape(P, t_all * F)
        per_core.append(dict(idx=idx_w, mask1=mask1, mask2=mask2, xg=xg))

    xown = []
    for c in range(NCORES):
        xo = np.zeros((NTP, F), np.float16)
        xo[:NT] = x16[c * NT : (c + 1) * NT]
        xown.append(xo)

    struct = dict(
        n=n,
        nO=nO.tolist(), nA=nA.tolist(), nB=nB_.tolist(),
        o_start=o_start.tolist(), a_start=a_start.tolist(),
        b_start=b_start.tolist(),
        tO_all=tO_all, tA_all=tA_all, tB_all=tB_all, t_all=t_all,
    )
    return struct, per_core, xown


# ----------------------------------------------------------------------------
# program builder
# ----------------------------------------------------------------------------

def _build(struct):
    t_all = struct["t_all"]
    tO_all = struct["tO_all"]
    tA_all = struct["tA_all"]
    nO, nA, nB_ = struct["nO"], struct["nA"], struct["nB"]
    o_start = struct["o_start"]
    a_start, b_start = struct["a_start"], struct["b_start"]
    starts3 = (o_start, a_start, b_start)
    ntile3 = (nO, nA, nB_)

    # gather batches: runs of <= BG tiles per region (0=own 1=A 2=B)
    tile2batch = {}
    region_batches = [[], [], []]
    batches = []
    for lo, hi, h in (
        (0, tO_all, 0),
        (tO_all, tO_all + tA_all, 1),
        (tO_all + tA_all, t_all, 2),
    ):
        t0 = lo
        while t0 < hi:
            cnt = min(BG, hi - t0)
            region_batches[h].append(len(batches))
            batches.append((t0, cnt, h))
            for j in range(cnt):
                tile2batch[t0 + j] = (len(batches) - 1, j)
            t0 += cnt

    # per-block consumption order (must match the host-side mask column
    # permutation in _preprocess)
    cons_order = []
    for tb in range(NB):
        for h in range(3):
            cons_order += [starts3[h][tb] + j for j in range(ntile3[h][tb])]

    nc = bacc.Bacc(None, target_bir_lowering=False, debug=False, num_swdge_queues=NQ)

    # ---- kernel I/O ----
    xg_t = nc.declare_dram_parameter("xg", [P, t_all * F], F16, isOutput=False)
    xown_t = nc.declare_dram_parameter("xown", [NTP, F], F16, isOutput=False)
    idx_t = nc.declare_dram_parameter("idx", [P, t_all * 8], I16, isOutput=False)
    mask1_t = nc.declare_dram_parameter("mask1", [P, t_all * P], F16, isOutput=False)
    mask2_t = nc.declare_dram_parameter("mask2", [P, t_all * P], F16, isOutput=False)
    w1_t = nc.declare_dram_parameter("w1", [F, K * FH], F16, isOutput=False)
    b1_t = nc.declare_dram_parameter("b1", [P, 2], F32, isOutput=False)
    w2_t = nc.declare_dram_parameter("w2", [P, K * 2 * F], F16, isOutput=False)
    b2_t = nc.declare_dram_parameter("b2", [P, F], F16, isOutput=False)
    ident_t = nc.declare_dram_parameter("ident", [P, P], F16, isOutput=False)
    nident_t = nc.declare_dram_parameter("nident", [P, P], F16, isOutput=False)
    out_t = nc.declare_dram_parameter("out", [NTP, F], F32, isOutput=True)

    # internal DRAM: 7 AG rounds (T1,T2,T3, y4, b3, b2, b1), chunked A/B,
    # plus a local copy of the own rows (ready before the collective lands;
    # the own-sourced gather tiles bridge the AG latency each hop)
    aginA = [nc.dram_tensor(f"aginA{i}", [RA, EF], F16) for i in range(7)]
    aginB = [nc.dram_tensor(f"aginB{i}", [RB, EF], F16) for i in range(7)]
    agoutA = [
        nc.dram_tensor(f"agoutA{i}", [GA, EF], F16, addr_space="Shared")
        for i in range(7)
    ]
    agoutB = [
        nc.dram_tensor(f"agoutB{i}", [GB, EF], F16, addr_space="Shared")
        for i in range(7)
    ]
    own_dram = [nc.dram_tensor(f"own{i}", [NTP, EF], F16) for i in range(7)]
    y_dram = [nc.dram_tensor(f"ydram{k}", [NTP, F], F16) for k in range(4)]

    with tile.TileContext(nc) as tc:
        import contextlib

        with contextlib.ExitStack() as ctx:
            consts = ctx.enter_context(tc.tile_pool(name="consts", bufs=1))
            gpool = ctx.enter_context(tc.tile_pool(name="gath", bufs=6))
            mpool = ctx.enter_context(tc.tile_pool(name="mask", bufs=4))
            pseg = ctx.enter_context(tc.tile_pool(name="pseg", bufs=3, space="PSUM"))
            ptp = ctx.enter_context(tc.tile_pool(name="ptp", bufs=2, space="PSUM"))
            pwp = ctx.enter_context(tc.tile_pool(name="pwp", bufs=2, space="PSUM"))
            pyt = ctx.enter_context(tc.tile_pool(name="pyt", bufs=1, space="PSUM"))
            feat = ctx.enter_context(tc.tile_pool(name="feat", bufs=4))
            fstream = ctx.enter_context(tc.tile_pool(name="fstream", bufs=2))
            big = ctx.enter_context(tc.tile_pool(name="big", bufs=1))
            wsb = ctx.enter_context(tc.tile_pool(name="wsb", bufs=4))

            # ---- load constants ----
            idx_sb = consts.tile([P, t_all * 8], I16)
            nc.sync.dma_start(out=idx_sb[:], in_=idx_t[:])
            w1_sb = consts.tile([F, K * FH], F16)
            nc.sync.dma_start(out=w1_sb[:], in_=w1_t[:])
            b1_sb = consts.tile([P, 2], F32)
            nc.sync.dma_start(out=b1_sb[:], in_=b1_t[:])
            w2_sb = consts.tile([P, K * 2 * F], F16)
            nc.sync.dma_start(out=w2_sb[:], in_=w2_t[:])
            b2_sb = consts.tile([P, F], F16)
            nc.sync.dma_start(out=b2_sb[:], in_=b2_t[:])
            ident_sb = consts.tile([P, P], F16)
            nc.sync.dma_start(out=ident_sb[:], in_=ident_t[:])
            nident_sb = consts.tile([P, P], F16)
            nc.sync.dma_start(out=nident_sb[:], in_=nident_t[:])

            def own_view(dram):
                return dram.ap().rearrange("(b p) f -> p b f", p=P)

            x_str = fstream.tile([P, NB, F], F16, tag="fs", name="x_str")
            nc.sync.dma_start(out=x_str[:], in_=own_view(xown_t))

            out1 = big.tile([P, NB, 2, P], F16, tag="out1")

            gq = [0]

            # batch issue order within a hop: own-sourced batches first (their
            # table is local, ready ~30us before the collectives land), then
            # chunk A with B interleaved after a short lead.
            LEAD = 4
            o_b, a_b, b_b = region_batches
            issue_order = list(o_b) + list(a_b[:LEAD])
            ia, ib = LEAD, 0
            while ia < len(a_b) or ib < len(b_b):
                if ib < len(b_b):
                    issue_order.append(b_b[ib]); ib += 1
                if ia < len(a_b):
                    issue_order.append(a_b[ia]); ia += 1

            MC = 8
            mchunks = [
                (i, cons_order[i : i + MC]) for i in range(0, len(cons_order), MC)
            ]

            def seg_prop(srcO, srcA, srcB, mask_t, extras, out_cb, ag_idx=None,
                         xg=None):
                """One hop. extras(tb) -> [(ident_ap, rhs_ap), ...] appended to
                each block's psum group. out_cb(tb, psum). ag_idx: AG round to
                ship (chunk A after block CA-1, chunk B at the end). xg: DRAM
                param with pre-gathered source rows (hop 1) — streamed
                sequentially instead of dma_gather."""
                srcs = (srcO, srcA, srcB)
                gbufs = {}
                mbufs = {}
                xgbufs = {}

                def issue(blist):
                    if xg is not None:
                        return
                    for bi in blist:
                        t0, cnt, h = batches[bi]
                        g = gpool.tile([P, BG, EF], F16, tag=f"gath{h}", name="g")
                        nc.gpsimd.dma_gather(
                            out_ap=g[:, :cnt, :],
                            in_ap=srcs[h][:, :],
                            idxs_ap=idx_sb[:, t0 * 8 : (t0 + cnt) * 8],
                            num_idxs=cnt * P,
                            num_idxs_reg=cnt * P,
                            elem_size=EF,
                            queue_num=gq[0] % NQ,
                        )
                        gq[0] += 1
                        gbufs[bi] = g

                def load_masks(lo, hi):
                    # mask chunks whose first tile's rank is in [lo, hi)
                    for i, chunk in mchunks:
                        if lo <= i < hi:
                            mt = mpool.tile(
                                [P, len(chunk) * P], F16, tag="mask", name="mt"
                            )
                            nc.sync.dma_start(
                                out=mt[:],
                                in_=mask_t[:, i * P : (i + len(chunk)) * P],
                            )
                            for j, t in enumerate(chunk):
                                mbufs[t] = (mt, j)
                            if xg is not None:
                                xt = mpool.tile(
                                    [P, len(chunk) * F], F16, tag="xg", name="xt"
                                )
                                nc.sync.dma_start(
                                    out=xt[:],
                                    in_=xg[:, i * F : (i + len(chunk)) * F],
                                )
                                for j, t in enumerate(chunk):
                                    xgbufs[t] = (xt, j)

                def blocks(lo, hi):
                    for tb in range(lo, hi):
                        tiles = []
                        for h in range(3):
                            tiles += [
                                starts3[h][tb] + j for j in range(ntile3[h][tb])
                            ]
                        ex = extras(tb) if extras else []
                        psum = pseg.tile([P, F], F32, tag="pseg", name="psum")
                        for ti, t in enumerate(tiles):
                            mt, mj = mbufs[t]
                            if xg is not None:
                                xt, xj = xgbufs[t]
                                rhs = xt[:, xj * F : (xj + 1) * F]
                            else:
                                bi, off = tile2batch[t]
                                rhs = gbufs[bi][:, off, 0:F]
                            last = ti == len(tiles) - 1 and not ex
                            nc.tensor.matmul(
                                out=psum[:],
                                lhsT=mt[:, mj * P : (mj + 1) * P],
                                rhs=rhs,
                                start=(ti == 0),
                                stop=last,
                            )
                        for xi, (idm, rhs) in enumerate(ex):
                            nc.tensor.matmul(
                                out=psum[:],
                                lhsT=idm[:],
                                rhs=rhs,
                                start=False,
                                stop=(xi == len(ex) - 1),
                            )
                        out_cb(tb, psum)

                # Unbroken gather stream; AG triggers after their producing
                # blocks (Tile deps are program-order — a consumer emitted
                # before its producer reads stale data).  The AG-A trigger
                # fires immediately once reached (blocks 0..CA-1 long done);
                # only its ~15us collective latency is exposed to the next
                # hop's first chunk-A gathers.
                issue(issue_order)
                load_masks(0, len(cons_order))
                blocks(0, CA)
                if ag_idx is not None:
                    do_own(ag_idx, 0)
                    do_ag(ag_idx, 0)
                blocks(CA, NB)
                if ag_idx is not None:
                    do_own(ag_idx, 1)
                    do_ag(ag_idx, 1)

            cur_tbl = {}

            def do_own(i, h):
                # local copy of own rows: ready well before the collective,
                # feeds the next hop's own-sourced bridge tiles
                src = cur_tbl[i]
                v = own_dram[i].ap().rearrange("(b p) f -> p b f", p=P)
                nblk = CA if h == 0 else NB - CA
                off = 0 if h == 0 else CA
                nc.scalar.dma_start(
                    out=v[:, off : off + nblk, 0:F],
                    in_=src[:, off : off + nblk, :],
                )

            def do_ag(i, h):
                src = cur_tbl[i]
                agin = aginA[i] if h == 0 else aginB[i]
                agout = agoutA[i] if h == 0 else agoutB[i]
                v = agin.ap().rearrange("(b p) f -> p b f", p=P)
                nblk = CA if h == 0 else NB - CA
                off = 0 if h == 0 else CA
                # scalar-engine HWDGE: queues behind the block copies this
                # depends on, keeping the sync queue free for mask streaming
                nc.scalar.dma_start(
                    out=v[:, :, 0:F], in_=src[:, off : off + nblk, :]
                )
                nc.gpsimd.collective_compute(
                    "AllGather",
                    ALU.bypass,
                    replica_groups=[list(range(NCORES))],
                    ins=[agin[:, :].opt()],
                    outs=[agout[:, :].opt()],
                )

            def w1_block(k, src, tb):
                tp = ptp.tile([F, P], F16, tag="tp", name="tp")
                nc.tensor.transpose(
                    out=tp[:], in_=src[:, tb, :], identity=ident_sb[:]
                )
                tfm = wsb.tile([F, P], F16, tag="tfm", name="tfm")
                nc.scalar.copy(out=tfm[:], in_=tp[:])
                for hh in range(2):
                    wp = pwp.tile([P, P], F32, tag="wp", name="wp")
                    nc.tensor.matmul(
                        out=wp[:],
                        lhsT=w1_sb[:, k * FH + hh * P : k * FH + (hh + 1) * P],
                        rhs=tfm[:],
                        start=True,
                        stop=True,
                    )
                    dst = out1[:, tb, hh, :]
                    if k == 0:
                        nc.scalar.copy(out=dst, in_=wp[:])
                    else:
                        nc.vector.tensor_tensor(
                            out=dst, in0=wp[:], in1=dst, op=ALU.add
                        )

            def w1_pass(k, src):
                for tb in range(NB):
                    w1_block(k, src, tb)

            def y_block(k, tb, ycur):
                yp = ptp.tile([F, P], F32, tag="tp", name="yp")
                for hh in range(2):
                    nc.tensor.matmul(
                        out=yp[:],
                        lhsT=w2_sb[:, (k * 2 + hh) * F : (k * 2 + hh + 1) * F],
                        rhs=out1[:, tb, hh, :],
                        start=(hh == 0),
                        stop=(hh == 1),
                    )
                yfm = wsb.tile([F, P], F16, tag="tfm", name="yfm")
                nc.scalar.copy(out=yfm[:], in_=yp[:])
                ytp = pyt.tile([P, F], F16, tag="ytp", name="ytp")
                nc.tensor.transpose(
                    out=ytp[:], in_=yfm[:], identity=ident_sb[:F, :F]
                )
                if k == 0:
                    nc.vector.tensor_tensor(
                        out=ycur[:, tb, :], in0=ytp[:], in1=b2_sb[:], op=ALU.add
                    )
                else:
                    nc.scalar.copy(out=ycur[:, tb, :], in_=ytp[:])

            # ---------------- layer 1 ----------------
            w1_pass(0, x_str)

            # feat pool rotation (bufs=4): t1,t2,t3,b4,t4,b3,b2,b1 pairs
            # each new tile with one whose lifetime has ended.
            b4 = None
            t_own = {0: x_str}
            for k in range(1, K):
                if k == K - 1:
                    b4 = feat.tile([P, NB, F], F16, tag="feat", name="b4")
                cur = feat.tile([P, NB, F], F16, tag="feat", name=f"t_own{k}")
                t_own[k] = cur
                if k == 1:
                    srcO, srcA, srcB, mt, xgp = None, None, None, mask1_t, xg_t
                else:
                    srcO = own_dram[k - 2]
                    srcA, srcB = agoutA[k - 2], agoutB[k - 2]
                    mt, xgp = mask2_t, None
                prev2 = t_own[k - 2] if k >= 2 else None

                def extras(tb, prev2=prev2):
                    if prev2 is None:
                        return []
                    return [(nident_sb, prev2[:, tb, 0:F])]

                if k < K - 1:
                    def rec(tb, psum, cur=cur):
                        nc.vector.tensor_copy(out=cur[:, tb, :], in_=psum[:])

                    cur_tbl[k - 1] = cur
                    seg_prop(srcO, srcA, srcB, mt, extras, rec,
                             ag_idx=k - 1, xg=xgp)
                    w1_pass(k, cur)
                else:
                    # hop 4 folded: T4 -> w1(4) -> relu -> y4 -> b4 per
                    # block, so y4's AG ships right after block CA-1 and the
                    # layer-1 -> layer-2 transition costs no serial phase.
                    def rec4(tb, psum, cur=cur):
                        nc.vector.tensor_copy(out=cur[:, tb, :], in_=psum[:])
                        w1_block(K - 1, cur, tb)
                        for hh in range(2):
                            sl = out1[:, tb, hh, :]
                            nc.scalar.activation(
                                out=sl, in_=sl, func=ACTF.Relu,
                                bias=b1_sb[:, hh : hh + 1],
                            )
                        y_block(4, tb, b4)

                    cur_tbl[3] = b4
                    seg_prop(srcO, srcA, srcB, mt, extras, rec4,
                             ag_idx=3, xg=xgp)

            # ---------------- y_k (k<4) while b3's gathers run ------------
            for k in (3, 2, 1, 0):
                ycur = fstream.tile([P, NB, F], F16, tag="fs", name=f"ycur{k}")
                for tb in range(NB):
                    y_block(k, tb, ycur)
                nc.sync.dma_start(out=own_view(y_dram[k]), in_=ycur[:])

            # ---------------- layer 2 (Clenshaw) ----------------
            b_own = {4: b4}
            for k, agi in ((3, 4), (2, 5), (1, 6)):
                cur = feat.tile([P, NB, F], F16, tag="feat", name=f"b_own{k}")
                b_own[k] = cur
                sub = b_own.get(k + 2)
                ystr = fstream.tile([P, NB, F], F16, tag="fs", name=f"ystr{k}")
                nc.sync.dma_start(out=ystr[:], in_=own_view(y_dram[k]))

                def extras(tb, sub=sub, yk=ystr):
                    ex = [(ident_sb, yk[:, tb, 0:F])]
                    if sub is not None:
                        ex.append((nident_sb, sub[:, tb, 0:F]))
                    return ex

                def rec(tb, psum, cur=cur):
                    nc.vector.tensor_copy(out=cur[:, tb, :], in_=psum[:])

                cur_tbl[agi] = cur
                seg_prop(own_dram[agi - 1], agoutA[agi - 1], agoutB[agi - 1],
                         mask2_t, extras, rec, ag_idx=agi)

            out_sb = big.tile([P, NB, F], F32, tag="outsb", name="out_sb")
            y0str = fstream.tile([P, NB, F], F16, tag="fs", name="y0str")
            nc.sync.dma_start(out=y0str[:], in_=own_view(y_dram[0]))

            def extras_fin(tb):
                return [
                    (ident_sb, y0str[:, tb, 0:F]),
                    (nident_sb, b_own[2][:, tb, 0:F]),
                ]

            def rec_fin(tb, psum):
                nc.vector.tensor_copy(out=out_sb[:, tb, :], in_=psum[:])

            seg_prop(own_dram[6], agoutA[6], agoutB[6], mask1_t, extras_fin,
                     rec_fin)
            nc.sync.dma_start(out=own_view(out_t), in_=out_sb[:])

    nc.finalize()
    return nc


# ----------------------------------------------------------------------------
# entry point
# ----------------------------------------------------------------------------

def _run(x, edge_index, train_edge_weight, W1, b1, W2, b2, trace=False):
    struct, per_core, xown = _preprocess(x, edge_index, train_edge_weight)
    nc = _build(struct)

    W1 = np.asarray(W1, np.float32)
    W2 = np.asarray(W2, np.float32)
    b1 = np.asarray(b1, np.float32)
    b2 = np.asarray(b2, np.float32)
    w1r = W1.transpose(1, 0, 2).reshape(F, K * FH).astype(np.float16).copy()
    b1r = b1.reshape(2, P).T.astype(np.float32).copy()
    w2r = (
        W2.reshape(K, 2, P, F).transpose(2, 0, 1, 3).reshape(P, K * 2 * F)
        .astype(np.float16).copy()
    )
    b2r = np.tile(b2[None, :], (P, 1)).astype(np.float16).copy()
    ident = np.eye(P, dtype=np.float16)
    nident = (-np.eye(P)).astype(np.float16)

    in_maps = []
    for c in range(NCORES):
        pc = per_core[c]
        in_maps.append(
            {
                "xg": pc["xg"], "xown": xown[c],
                "idx": pc["idx"], "mask1": pc["mask1"], "mask2": pc["mask2"],
                "w1": w1r, "b1": b1r, "w2": w2r, "b2": b2r,
                "ident": ident, "nident": nident,
            }
        )
    res = run_bass_kernel_spmd(
        nc, in_maps, core_ids=list(range(NCORES)), trace=trace
    )
    n = struct["n"]
    out = np.empty((n, F), np.float32)
    for c in range(NCORES):
        out[c * NT : (c + 1) * NT] = res.results[c]["out"][:NT]
    if trace:
        return out, res.exec_time_ns
    return out


def kernel(x, edge_index, train_edge_weight, W1, b1, W2, b2):
    trace = bool(os.environ.get("GNN_TRACE"))
    r = _run(x, edge_index, train_edge_weight, W1, b1, W2, b2, trace=trace)
    if trace:
        out, t = r
        print(f"HW exec time: {t} ns")
        return out
    return r


# revision 3
# speedup vs baseline: 1.0715x; 1.0715x over previous
"""ChebConv (K=5) x2 GNN decoder on 8 TRN2 NeuronCores — v2.

Key design points vs v1:
- fp16 node tables (rows padded to 128 halfs = 256B for the gather-elem
  constraint), fp16 weights/matmuls (FWL + 2x PE rate), f32 PSUM.
- dma_gather calls round-robin over all 4 SWDGE queues (4 Q7 core pairs +
  4 descriptor rings): measured 2.3us per 1024-idx call vs 8.6us on one
  queue. This was the v1 bottleneck (86% GpSimd occupancy).
- One-hot (edge->target, scaled by norm) matrices precomputed on host in
  fp16 and streamed from DRAM, removing all per-tile DVE work.
- Chebyshev / Clenshaw recurrence terms folded into the PE accumulation
  group via +-identity matmuls; psum evacuation on the Scalar engine.
- AllGather split into two chunks (each core's first 25 / last 24 blocks)
  so the first chunk ships while the second half of the hop computes, and
  the next hop's gathers start after only the matching chunk lands.
"""
import os
import sys

sys.path.insert(0, "/opt/trn_rl_repo")

import numpy as np


def _install_ntff_hook():
    import types

    if "antenv.axon_hooks" in sys.modules:
        return
    try:
        import antenv
        from trn_agent_boot.trn_boot import _ntff_profile_via_ctypes
    except Exception:
        return
    mod = types.ModuleType("antenv.axon_hooks")
    state = {"hook": None}
    mod.set_axon_ntff_profile_hook = lambda h: state.__setitem__("hook", h)
    mod.get_axon_ntff_profile_hook = lambda: state["hook"]
    sys.modules["antenv.axon_hooks"] = mod
    antenv.axon_hooks = mod
    try:
        hook = _ntff_profile_via_ctypes("/opt/axon/libaxon_pjrt.so")
        if hook is not None:
            mod.set_axon_ntff_profile_hook(hook)
    except Exception:
        pass


_install_ntff_hook()

import concourse.bass as bass
import concourse.bacc as bacc
import concourse.mybir as mybir
import concourse.tile as tile
from concourse.bass_utils import run_bass_kernel_spmd

F32 = mybir.dt.float32
F16 = mybir.dt.float16
I16 = mybir.dt.int16
ALU = mybir.AluOpType
ACTF = mybir.ActivationFunctionType

NCORES = 8
F = 64
FH = 256
K = 5
P = 128
NT = 6250            # owned real targets per core
NB = 49              # 128-blocks per core
NTP = NB * P         # 6272 padded rows per core
CA = 25              # chunk-A blocks
RA = CA * P          # 3200 chunk-A rows per core
RB = NTP - RA        # 3072 chunk-B rows per core
GA = NCORES * RA     # 25600 global chunk-A table rows
GB = NCORES * RB     # 24576
EF = 128             # padded fp16 row elements (256B)
BG = 8               # tiles per gather call (1024 idx HW cap)
NQ = 4               # SWDGE queues


# ----------------------------------------------------------------------------
# host-side preprocessing
# ----------------------------------------------------------------------------

def _preprocess(x, edge_index, w):
    n = x.shape[0]
    assert n == NCORES * NT

    row = np.asarray(edge_index[0], dtype=np.int64)
    col = np.asarray(edge_index[1], dtype=np.int64)
    w = np.asarray(w, dtype=np.float32)

    deg = np.zeros(n, np.float64)
    np.add.at(deg, row, w.astype(np.float64))
    dis = np.where(deg > 0, 1.0 / np.sqrt(np.maximum(deg, 1e-30)), 0.0)
    norm = (-dis[row] * w.astype(np.float64) * dis[col]).astype(np.float32)

    c_src = row // NT
    j_src = row % NT
    in_b = j_src >= RA

    core_t = col // NT
    loc = col % NT
    blk = loc // P
    cloc = (loc % P).astype(np.int64)

    # per-core 3-way source chunk: 0 = own core (local table, ready before
    # the AllGather lands -> bridges AG latency), 1 = global chunk A,
    # 2 = global chunk B
    edir = {}
    cnt = np.zeros((NCORES, NB, 3), np.int64)
    for c in range(NCORES):
        ei = np.nonzero(core_t == c)[0]
        ch = np.where(in_b[ei], 2, 1)  # own-bridge class disabled
        key = blk[ei] * 3 + ch
        order = np.argsort(key, kind="stable")
        ei = ei[order]
        bounds = np.searchsorted(key[order], np.arange(3 * NB + 1))
        edir[c] = (ei, bounds)
        cnt[c] = (bounds[1:] - bounds[:-1]).reshape(NB, 3)

    ntile = (cnt.max(axis=0) + P - 1) // P  # [NB, 3]
    ntile[:, 1:] = np.maximum(ntile[:, 1:], 1)
    nO, nA, nB_ = ntile[:, 0], ntile[:, 1], ntile[:, 2]
    tO_all = int(nO.sum())
    tA_all = int(nA.sum())
    tB_all = int(nB_.sum())
    t_all = tO_all + tA_all + tB_all
    o_start = np.concatenate([[0], np.cumsum(nO)])[:-1]
    a_start = tO_all + np.concatenate([[0], np.cumsum(nA)])[:-1]
    b_start = tO_all + tA_all + np.concatenate([[0], np.cumsum(nB_)])[:-1]
    starts3 = (o_start, a_start, b_start)

    per_core = []
    x16 = np.asarray(x, np.float32).astype(np.float16)
    for c in range(NCORES):
        ei, bounds = edir[c]
        idx_flat = np.zeros(t_all * P, np.int16)
        cloc_flat = np.zeros(t_all * P, np.int64)
        nrm_flat = np.zeros(t_all * P, np.float32)
        src_flat = np.zeros(t_all * P, np.int64)
        for tb in range(NB):
            for h in range(3):
                lo, hi = bounds[tb * 3 + h], bounds[tb * 3 + h + 1]
                sel = ei[lo:hi]
                o = starts3[h][tb] * P
                s = row[sel]
                if h == 0:
                    idx_flat[o : o + len(sel)] = (s % NT).astype(np.int16)
                elif h == 1:
                    idx_flat[o : o + len(sel)] = (
                        (s // NT) * RA + (s % NT)
                    ).astype(np.int16)
                else:
                    idx_flat[o : o + len(sel)] = (
                        (s // NT) * RB + (s % NT - RA)
                    ).astype(np.int16)
                cloc_flat[o : o + len(sel)] = cloc[sel]
                nrm_flat[o : o + len(sel)] = norm[sel]
                src_flat[o : o + len(sel)] = s
        iw = idx_flat.reshape(t_all, 8, 16).transpose(2, 0, 1).reshape(16, t_all * 8)
        idx_w = np.tile(iw, (8, 1))
        # one-hot masks [128 edge-slot partitions, t_all*128 target cols],
        # tile-columns permuted into per-block consumption order so mask
        # chunks stream as single contiguous DMAs.
        cons = []
        for tb in range(NB):
            for h in range(3):
                cons += [starts3[h][tb] + j for j in range(ntile[tb, h])]
        m = np.zeros((t_all * P, P), np.float16)
        m[np.arange(t_all * P), cloc_flat] = nrm_flat
        m = m.reshape(t_all, P, P).transpose(1, 0, 2)  # [P, t_all, P]
        mask1 = np.ascontiguousarray(m[:, cons, :]).reshape(P, t_all * P)
        mask2 = (mask1.astype(np.float32) * 2.0).astype(np.float16)
        xg = x16[src_flat]  # [t_all*P, F] fp16, junk on pad slots (mask=0)
        xg = xg.reshape(t_all, P, F).transpose(1, 0, 2)  # [P, t_all, F]
        xg = np.ascontiguousarray(xg[:, cons, :]).reshape(P, t_all * F)
        per_core.append(dict(idx=idx_w, mask1=mask1, mask2=mask2, xg=xg))

    xown = []
    for c in range(NCORES):
        xo = np.zeros((NTP, F), np.float16)
        xo[:NT] = x16[c * NT : (c + 1) * NT]
        xown.append(xo)

    struct = dict(
        n=n,
        nO=nO.tolist(), nA=nA.tolist(), nB=nB_.tolist(),
        o_start=o_start.tolist(), a_start=a_start.tolist(),
        b_start=b_start.tolist(),
        tO_all=tO_all, tA_all=tA_all, tB_all=tB_all, t_all=t_all,
    )
    return struct, per_core, xown


# ----------------------------------------------------------------------------
# program builder
# ----------------------------------------------------------------------------

def _build(struct):
    t_all = struct["t_all"]
    tO_all = struct["tO_all"]
    tA_all = struct["tA_all"]
    nO, nA, nB_ = struct["nO"], struct["nA"], struct["nB"]
    o_start = struct["o_start"]
    a_start, b_start = struct["a_start"], struct["b_start"]
    starts3 = (o_start, a_start, b_start)
    ntile3 = (nO, nA, nB_)

    # gather batches: runs of <= BG tiles per region (0=own 1=A 2=B)
    tile2batch = {}
    region_batches = [[], [], []]
    batches = []
    for lo, hi, h in (
        (0, tO_all, 0),
        (tO_all, tO_all + tA_all, 1),
        (tO_all + tA_all, t_all, 2),
    ):
        t0 = lo
        while t0 < hi:
            cnt = min(BG, hi - t0)
            region_batches[h].append(len(batches))
            batches.append((t0, cnt, h))
            for j in range(cnt):
                tile2batch[t0 + j] = (len(batches) - 1, j)
            t0 += cnt

    # per-block consumption order (must match the host-side mask column
    # permutation in _preprocess)
    cons_order = []
    for tb in range(NB):
        for h in range(3):
            cons_order += [starts3[h][tb] + j for j in range(ntile3[h][tb])]

    nc = bacc.Bacc(None, target_bir_lowering=False, debug=False, num_swdge_queues=NQ)

    # ---- kernel I/O ----
    xg_t = nc.declare_dram_parameter("xg", [P, t_all * F], F16, isOutput=False)
    xown_t = nc.declare_dram_parameter("xown", [NTP, F], F16, isOutput=False)
    idx_t = nc.declare_dram_parameter("idx", [P, t_all * 8], I16, isOutput=False)
    mask1_t = nc.declare_dram_parameter("mask1", [P, t_all * P], F16, isOutput=False)
    mask2_t = nc.declare_dram_parameter("mask2", [P, t_all * P], F16, isOutput=False)
    w1_t = nc.declare_dram_parameter("w1", [F, K * FH], F16, isOutput=False)
    b1_t = nc.declare_dram_parameter("b1", [P, 2], F32, isOutput=False)
    w2_t = nc.declare_dram_parameter("w2", [P, K * 2 * F], F16, isOutput=False)
    b2_t = nc.declare_dram_parameter("b2", [P, F], F16, isOutput=False)
    ident_t = nc.declare_dram_parameter("ident", [P, P], F16, isOutput=False)
    nident_t = nc.declare_dram_parameter("nident", [P, P], F16, isOutput=False)
    out_t = nc.declare_dram_parameter("out", [NTP, F], F32, isOutput=True)

    # internal DRAM: 7 AG rounds (T1,T2,T3, y4, b3, b2, b1), chunked A/B,
    # plus a local copy of the own rows (ready before the collective lands;
    # the own-sourced gather tiles bridge the AG latency each hop)
    aginA = [nc.dram_tensor(f"aginA{i}", [RA, EF], F16) for i in range(7)]
    aginB = [nc.dram_tensor(f"aginB{i}", [RB, EF], F16) for i in range(7)]
    agoutA = [
        nc.dram_tensor(f"agoutA{i}", [GA, EF], F16, addr_space="Shared")
        for i in range(7)
    ]
    agoutB = [
        nc.dram_tensor(f"agoutB{i}", [GB, EF], F16, addr_space="Shared")
        for i in range(7)
    ]
    own_dram = [nc.dram_tensor(f"own{i}", [NTP, EF], F16) for i in range(7)]
    y_dram = [nc.dram_tensor(f"ydram{k}", [NTP, F], F16) for k in range(4)]

    with tile.TileContext(nc) as tc:
        import contextlib

        with contextlib.ExitStack() as ctx:
            consts = ctx.enter_context(tc.tile_pool(name="consts", bufs=1))
            gpool = ctx.enter_context(tc.tile_pool(name="gath", bufs=6))
            mpool = ctx.enter_context(tc.tile_pool(name="mask", bufs=4))
            pseg = ctx.enter_context(tc.tile_pool(name="pseg", bufs=3, space="PSUM"))
            ptp = ctx.enter_context(tc.tile_pool(name="ptp", bufs=2, space="PSUM"))
            pwp = ctx.enter_context(tc.tile_pool(name="pwp", bufs=2, space="PSUM"))
            pyt = ctx.enter_context(tc.tile_pool(name="pyt", bufs=1, space="PSUM"))
            feat = ctx.enter_context(tc.tile_pool(name="feat", bufs=4))
            fstream = ctx.enter_context(tc.tile_pool(name="fstream", bufs=2))
            big = ctx.enter_context(tc.tile_pool(name="big", bufs=1))
            wsb = ctx.enter_context(tc.tile_pool(name="wsb", bufs=4))

            # ---- load constants ----
            idx_sb = consts.tile([P, t_all * 8], I16)
            nc.sync.dma_start(out=idx_sb[:], in_=idx_t[:])
            w1_sb = consts.tile([F, K * FH], F16)
            nc.sync.dma_start(out=w1_sb[:], in_=w1_t[:])
            b1_sb = consts.tile([P, 2], F32)
            nc.sync.dma_start(out=b1_sb[:], in_=b1_t[:])
            w2_sb = consts.tile([P, K * 2 * F], F16)
            nc.sync.dma_start(out=w2_sb[:], in_=w2_t[:])
            b2_sb = consts.tile([P, F], F16)
            nc.sync.dma_start(out=b2_sb[:], in_=b2_t[:])
            ident_sb = consts.tile([P, P], F16)
            nc.sync.dma_start(out=ident_sb[:], in_=ident_t[:])
            nident_sb = consts.tile([P, P], F16)
            nc.sync.dma_start(out=nident_sb[:], in_=nident_t[:])

            def own_view(dram):
                return dram.ap().rearrange("(b p) f -> p b f", p=P)

            x_str = fstream.tile([P, NB, F], F16, tag="fs", name="x_str")
            nc.sync.dma_start(out=x_str[:], in_=own_view(xown_t))

            out1 = big.tile([P, NB, 2, P], F16, tag="out1")

            gq = [0]

            # batch issue order within a hop: plain A/B interleave (matches
            # the per-block A-then-B consumption order)
            o_b, a_b, b_b = region_batches
            issue_order = list(o_b)
            ia, ib = 0, 0
            while ia < len(a_b) or ib < len(b_b):
                if ia < len(a_b):
                    issue_order.append(a_b[ia]); ia += 1
                if ib < len(b_b):
                    issue_order.append(b_b[ib]); ib += 1

            MC = 8
            mchunks = [
                (i, cons_order[i : i + MC]) for i in range(0, len(cons_order), MC)
            ]

            def seg_prop(srcO, srcA, srcB, mask_t, extras, out_cb, ag_idx=None,
                         xg=None):
                """One hop. extras(tb) -> [(ident_ap, rhs_ap), ...] appended to
                each block's psum group. out_cb(tb, psum). ag_idx: AG round to
                ship (chunk A after block CA-1, chunk B at the end). xg: DRAM
                param with pre-gathered source rows (hop 1) — streamed
                sequentially instead of dma_gather."""
                srcs = (srcO, srcA, srcB)
                gbufs = {}
                mbufs = {}
                xgbufs = {}

                def issue(blist):
                    if xg is not None:
                        return
                    for bi in blist:
                        t0, cnt, h = batches[bi]
                        g = gpool.tile([P, BG, EF], F16, tag=f"gath{h}", name="g")
                        nc.gpsimd.dma_gather(
                            out_ap=g[:, :cnt, :],
                            in_ap=srcs[h][:, :],
                            idxs_ap=idx_sb[:, t0 * 8 : (t0 + cnt) * 8],
                            num_idxs=cnt * P,
                            num_idxs_reg=cnt * P,
                            elem_size=EF,
                            queue_num=gq[0] % NQ,
                        )
                        gq[0] += 1
                        gbufs[bi] = g

                def load_masks(lo, hi):
                    # mask chunks whose first tile's rank is in [lo, hi)
                    for i, chunk in mchunks:
                        if lo <= i < hi:
                            mt = mpool.tile(
                                [P, len(chunk) * P], F16, tag="mask", name="mt"
                            )
                            nc.sync.dma_start(
                                out=mt[:],
                                in_=mask_t[:, i * P : (i + len(chunk)) * P],
                            )
                            for j, t in enumerate(chunk):
                                mbufs[t] = (mt, j)
                            if xg is not None:
                                xt = mpool.tile(
                                    [P, len(chunk) * F], F16, tag="xg", name="xt"
                                )
                                nc.sync.dma_start(
                                    out=xt[:],
                                    in_=xg[:, i * F : (i + len(chunk)) * F],
                                )
                                for j, t in enumerate(chunk):
                                    xgbufs[t] = (xt, j)

                def blocks(lo, hi):
                    for tb in range(lo, hi):
                        tiles = []
                        for h in range(3):
                            tiles += [
                                starts3[h][tb] + j for j in range(ntile3[h][tb])
                            ]
                        ex = extras(tb) if extras else []
                        psum = pseg.tile([P, F], F32, tag="pseg", name="psum")
                        for ti, t in enumerate(tiles):
                            mt, mj = mbufs[t]
                            if xg is not None:
                                xt, xj = xgbufs[t]
                                rhs = xt[:, xj * F : (xj + 1) * F]
                            else:
                                bi, off = tile2batch[t]
                                rhs = gbufs[bi][:, off, 0:F]
                            last = ti == len(tiles) - 1 and not ex
                            nc.tensor.matmul(
                                out=psum[:],
                                lhsT=mt[:, mj * P : (mj + 1) * P],
                                rhs=rhs,
                                start=(ti == 0),
                                stop=last,
                            )
                        for xi, (idm, rhs) in enumerate(ex):
                            nc.tensor.matmul(
                                out=psum[:],
                                lhsT=idm[:],
                                rhs=rhs,
                                start=False,
                                stop=(xi == len(ex) - 1),
                            )
                        out_cb(tb, psum)

                # Unbroken gather stream; AG triggers after their producing
                # blocks (Tile deps are program-order — a consumer emitted
                # before its producer reads stale data).  The AG-A trigger
                # fires immediately once reached (blocks 0..CA-1 long done);
                # only its ~15us collective latency is exposed to the next
                # hop's first chunk-A gathers.
                issue(issue_order)
                load_masks(0, len(cons_order))
                blocks(0, CA)
                if ag_idx is not None:
                    do_ag(ag_idx, 0)
                blocks(CA, NB)
                if ag_idx is not None:
                    do_ag(ag_idx, 1)

            cur_tbl = {}

            def do_own(i, h):
                # local copy of own rows: ready well before the collective,
                # feeds the next hop's own-sourced bridge tiles
                src = cur_tbl[i]
                v = own_dram[i].ap().rearrange("(b p) f -> p b f", p=P)
                nblk = CA if h == 0 else NB - CA
                off = 0 if h == 0 else CA
                nc.scalar.dma_start(
                    out=v[:, off : off + nblk, 0:F],
                    in_=src[:, off : off + nblk, :],
                )

            def do_ag(i, h):
                src = cur_tbl[i]
                agin = aginA[i] if h == 0 else aginB[i]
                agout = agoutA[i] if h == 0 else agoutB[i]
                v = agin.ap().rearrange("(b p) f -> p b f", p=P)
                nblk = CA if h == 0 else NB - CA
                off = 0 if h == 0 else CA
                # scalar-engine HWDGE: queues behind the block copies this
                # depends on, keeping the sync queue free for mask streaming
                nc.scalar.dma_start(
                    out=v[:, :, 0:F], in_=src[:, off : off + nblk, :]
                )
                nc.gpsimd.collective_compute(
                    "AllGather",
                    ALU.bypass,
                    replica_groups=[list(range(NCORES))],
                    ins=[agin[:, :].opt()],
                    outs=[agout[:, :].opt()],
                )

            def w1_block(k, src, tb):
                tp = ptp.tile([F, P], F16, tag="tp", name="tp")
                nc.tensor.transpose(
                    out=tp[:], in_=src[:, tb, :], identity=ident_sb[:]
                )
                tfm = wsb.tile([F, P], F16, tag="tfm", name="tfm")
                nc.scalar.copy(out=tfm[:], in_=tp[:])
                for hh in range(2):
                    wp = pwp.tile([P, P], F32, tag="wp", name="wp")
                    nc.tensor.matmul(
                        out=wp[:],
                        lhsT=w1_sb[:, k * FH + hh * P : k * FH + (hh + 1) * P],
                        rhs=tfm[:],
                        start=True,
                        stop=True,
                    )
                    dst = out1[:, tb, hh, :]
                    if k == 0:
                        nc.scalar.copy(out=dst, in_=wp[:])
                    else:
                        nc.vector.tensor_tensor(
                            out=dst, in0=wp[:], in1=dst, op=ALU.add
                        )

            def w1_pass(k, src):
                for tb in range(NB):
                    w1_block(k, src, tb)

            def y_block(k, tb, ycur):
                yp = ptp.tile([F, P], F32, tag="tp", name="yp")
                for hh in range(2):
                    nc.tensor.matmul(
                        out=yp[:],
                        lhsT=w2_sb[:, (k * 2 + hh) * F : (k * 2 + hh + 1) * F],
                        rhs=out1[:, tb, hh, :],
                        start=(hh == 0),
                        stop=(hh == 1),
                    )
                yfm = wsb.tile([F, P], F16, tag="tfm", name="yfm")
                nc.scalar.copy(out=yfm[:], in_=yp[:])
                ytp = pyt.tile([P, F], F16, tag="ytp", name="ytp")
                nc.tensor.transpose(
                    out=ytp[:], in_=yfm[:], identity=ident_sb[:F, :F]
                )
                if k == 0:
                    nc.vector.tensor_tensor(
                        out=ycur[:, tb, :], in0=ytp[:], in1=b2_sb[:], op=ALU.add
                    )
                else:
                    nc.scalar.copy(out=ycur[:, tb, :], in_=ytp[:])

            # ---------------- layer 1 ----------------
            w1_pass(0, x_str)

            # feat pool rotation (bufs=4): t1,t2,t3,b4,t4,b3,b2,b1 pairs
            # each new tile with one whose lifetime has ended.
            b4 = None
            t_own = {0: x_str}
            for k in range(1, K):
                if k == K - 1:
                    b4 = feat.tile([P, NB, F], F16, tag="feat", name="b4")
                cur = feat.tile([P, NB, F], F16, tag="feat", name=f"t_own{k}")
                t_own[k] = cur
                if k == 1:
                    srcO, srcA, srcB, mt, xgp = None, None, None, mask1_t, xg_t
                else:
                    srcO = own_dram[k - 2]
                    srcA, srcB = agoutA[k - 2], agoutB[k - 2]
                    mt, xgp = mask2_t, None
                prev2 = t_own[k - 2] if k >= 2 else None

                def extras(tb, prev2=prev2):
                    if prev2 is None:
                        return []
                    return [(nident_sb, prev2[:, tb, 0:F])]

                def rec(tb, psum, cur=cur):
                    nc.vector.tensor_copy(out=cur[:, tb, :], in_=psum[:])

                cur_tbl[k - 1] = cur
                seg_prop(srcO, srcA, srcB, mt, extras, rec,
                         ag_idx=(k - 1) if k < K - 1 else None, xg=xgp)
                w1_pass(k, cur)

            # ---------------- layer 1 -> 2: relu ----------------
            for tb in range(NB):
                for hh in range(2):
                    sl = out1[:, tb, hh, :]
                    nc.scalar.activation(
                        out=sl, in_=sl, func=ACTF.Relu, bias=b1_sb[:, hh : hh + 1]
                    )

            # ---------------- y_k = relu(out1) @ W2[k] ----------------
            for k in (4, 3, 2, 1, 0):
                ycur = b4 if k == 4 else fstream.tile(
                    [P, NB, F], F16, tag="fs", name=f"ycur{k}"
                )
                for tb in range(NB):
                    y_block(k, tb, ycur)
                    if k == 4 and tb == CA - 1:
                        cur_tbl[3] = ycur
                        do_ag(3, 0)
                if k == 4:
                    do_ag(3, 1)
                else:
                    nc.sync.dma_start(out=own_view(y_dram[k]), in_=ycur[:])

            # ---------------- layer 2 (Clenshaw) ----------------
            b_own = {4: b4}
            for k, agi in ((3, 4), (2, 5), (1, 6)):
                cur = feat.tile([P, NB, F], F16, tag="feat", name=f"b_own{k}")
                b_own[k] = cur
                sub = b_own.get(k + 2)
                ystr = fstream.tile([P, NB, F], F16, tag="fs", name=f"ystr{k}")
                nc.sync.dma_start(out=ystr[:], in_=own_view(y_dram[k]))

                def extras(tb, sub=sub, yk=ystr):
                    ex = [(ident_sb, yk[:, tb, 0:F])]
                    if sub is not None:
                        ex.append((nident_sb, sub[:, tb, 0:F]))
                    return ex

                def rec(tb, psum, cur=cur):
                    nc.vector.tensor_copy(out=cur[:, tb, :], in_=psum[:])

                cur_tbl[agi] = cur
                seg_prop(own_dram[agi - 1], agoutA[agi - 1], agoutB[agi - 1],
                         mask2_t, extras, rec, ag_idx=agi)

            out_sb = big.tile([P, NB, F], F32, tag="outsb", name="out_sb")
            y0str = fstream.tile([P, NB, F], F16, tag="fs", name="y0str")
            nc.sync.dma_start(out=y0str[:], in_=own_view(y_dram[0]))

            def extras_fin(tb):
                return [
                    (ident_sb, y0str[:, tb, 0:F]),
                    (nident_sb, b_own[2][:, tb, 0:F]),
                ]

            def rec_fin(tb, psum):
                nc.vector.tensor_copy(out=out_sb[:, tb, :], in_=psum[:])

            seg_prop(own_dram[6], agoutA[6], agoutB[6], mask1_t, extras_fin,
                     rec_fin)
            nc.sync.dma_start(out=own_view(out_t), in_=out_sb[:])

    nc.finalize()
    return nc


# ----------------------------------------------------------------------------
# entry point
# ----------------------------------------------------------------------------

def _run(x, edge_index, train_edge_weight, W1, b1, W2, b2, trace=False):
    struct, per_core, xown = _preprocess(x, edge_index, train_edge_weight)
    nc = _build(struct)

    W1 = np.asarray(W1, np.float32)
    W2 = np.asarray(W2, np.float32)
    b1 = np.asarray(b1, np.float32)
    b2 = np.asarray(b2, np.float32)
    w1r = W1.transpose(1, 0, 2).reshape(F, K * FH).astype(np.float16).copy()
    b1r = b1.reshape(2, P).T.astype(np.float32).copy()
    w2r = (
        W2.reshape(K, 2, P, F).transpose(2, 0, 1, 3).reshape(P, K * 2 * F)
        .astype(np.float16).copy()
    )
    b2r = np.tile(b2[None, :], (P, 1)).astype(np.float16).copy()
    ident = np.eye(P, dtype=np.float16)
    nident = (-np.eye(P)).astype(np.float16)

    in_maps = []
    for c in range(NCORES):
        pc = per_core[c]
        in_maps.append(
            {
                "xg": pc["xg"], "xown": xown[c],
                "idx": pc["idx"], "mask1": pc["mask1"], "mask2": pc["mask2"],
                "w1": w1r, "b1": b1r, "w2": w2r, "b2": b2r,
                "ident": ident, "nident": nident,
            }
        )
    res = run_bass_kernel_spmd(
        nc, in_maps, core_ids=list(range(NCORES)), trace=trace
    )
    n = struct["n"]
    out = np.empty((n, F), np.float32)
    for c in range(NCORES):
        out[c * NT : (c + 1) * NT] = res.results[c]["out"][:NT]
    if trace:
        return out, res.exec_time_ns
    return out


def kernel(x, edge_index, train_edge_weight, W1, b1, W2, b2):
    trace = bool(os.environ.get("GNN_TRACE"))
    r = _run(x, edge_index, train_edge_weight, W1, b1, W2, b2, trace=trace)
    if trace:
        out, t = r
        print(f"HW exec time: {t} ns")
        return out
    return r
